# revision 12
# baseline (speedup 1.0000x reference)
"""Trainium2 Bass kernel for nn_HadamardTransform: out = value @ (weight + permutation).

weight is the scaled Sylvester Hadamard H4096/64, permutation is one-hot.
Data-parallel over token rows across 8 cores; each core computes, in the
transposed frame, o[n, m] = sum_k Hs[k, n] vT[k, m] + vT[src[n], m].

Decomposition Hs = H8 (x) H512 (Sylvester Kronecker):
  - PE: per k1-group i1 in 0..7, U_i1[n2, m] = sum_k2 (H512/64)[k2, n2] vT[i1*512+k2, m]
        (bf16 lhsT/rhs, fp32 PSUM, 4 accumulating matmuls per tile)
  - Act: PSUM -> SBUF drain with fp32 -> bf16 downcast (2-bank copies)
  - DVE: 3 radix-2 butterfly stages over the i1 axis in bf16 (2x DVE mode);
         one op per block offloaded to Pool.
  - Pool/SWDGE: per-output-row indirect gathers vT[src[n], m0:m0+512] fused
    with the final add via DMA compute (cce add).  The HW indirect-DMA path
    only supports one index per partition (2-D dest APs), so gathers are
    issued per j1 row.
Everything is bf16 on the wire: ~24.5 MB of DMA per core vs 52 MB for fp32,
which puts the kernel at the 360 GB/s-per-core DMA roofline (~68 us) with
PE (~57 us) and DVE (~52 us) hidden underneath.
"""

import sys

sys.path.insert(0, "/opt/trn_rl_repo")

import ml_dtypes
import numpy as np

import concourse.bacc as bacc
import concourse.bass as bass
import concourse.mybir as mybir
import concourse.tile as tile
from concourse.bass_utils import run_bass_kernel_spmd

ROWS = 8192
N = 4096
N_CORES = 8
MPC = ROWS // N_CORES  # 1024 token rows per core
KT = N // 128  # 32 k-tiles
NB = N // 128  # 32 n-blocks
MC = MPC // 512  # 2 m-chunks

I1 = 8  # radix handled by DVE butterflies
B = N // I1  # 512-point transform on the PE
KS = B // 128  # 4 k-subtiles per i1 group
J2B = B // 128  # 4 n2 blocks

BF16 = mybir.dt.bfloat16

_cache = {}


def _hadamard_pm1(n):
    idx = np.arange(n, dtype=np.int64)
    m = idx[:, None] & idx[None, :]
    pop = np.zeros_like(m)
    for _ in range(int(np.log2(n))):
        pop += m & 1
        m >>= 1
    return np.where(pop % 2 == 0, 1.0, -1.0).astype(np.float32)


def check_structure(weight, permutation):
    """weight must be the scaled Sylvester Hadamard, permutation one-hot."""
    H = _hadamard_pm1(N) / np.sqrt(np.float32(N))
    if not np.array_equal(weight, H):
        return None
    src = np.argmax(permutation, axis=0).astype(np.int32)
    ok = (
        permutation[src, np.arange(N)].min() == 1.0
        and permutation.sum() == N
        and np.abs(permutation).sum() == N
    )
    return src if ok else None


def build_hadamard(reps=1, hw_loop=False):
    nc = bacc.Bacc("TRN2", target_bir_lowering=False)
    vT = nc.dram_tensor("vT", (N, MPC), BF16, kind="ExternalInput")
    hc = nc.dram_tensor("hc", (B, B), BF16, kind="ExternalInput")
    gidx = nc.dram_tensor("gidx", (128, J2B, I1), mybir.dt.int32, kind="ExternalInput")
    o = nc.dram_tensor("o", (N, MPC), BF16, kind="ExternalOutput")

    add, sub = mybir.AluOpType.add, mybir.AluOpType.subtract

    with tile.TileContext(nc) as tc:
        with (
            tc.tile_pool(name="h", bufs=1) as h_pool,
            tc.tile_pool(name="gi", bufs=1) as gi_pool,
            tc.tile_pool(name="vt", bufs=1) as vt_pool,
            tc.tile_pool(name="ps", bufs=4, space="PSUM") as ps_pool,
            tc.tile_pool(name="u", bufs=3) as u_pool,
            tc.tile_pool(name="a", bufs=3) as a_pool,
            tc.tile_pool(name="c", bufs=3) as c_pool,
        ):
            # H and index loads go on the Pool queue (Act's first instruction
            # triggers a slow activation-table load; SP streams value.T at t=0)
            hts = []
            for ks in range(KS):
                ht = h_pool.tile([128, B], BF16, tag=f"h{ks}")
                nc.gpsimd.dma_start(out=ht, in_=hc[ks * 128 : (ks + 1) * 128, :])
                hts.append(ht)
            gi = gi_pool.tile([128, J2B, I1], mybir.dt.int32, tag="gi")
            nc.gpsimd.dma_start(out=gi, in_=gidx[:, :, :])

            # warm the PE out of its p-state ramp while the input streams in;
            # the warmup writes into block 0's first PSUM tile (overwritten by
            # the real accumulation) so no PSUM rotation slot is wasted
            wps = ps_pool.tile([128, 2, 512], mybir.dt.float32, tag="ps")
            for w in range(7):
                nc.tensor.matmul(
                    out=wps[:, 0, :],
                    lhsT=hts[0][:, 0:128],
                    rhs=hts[0][:, :],
                    start=True,
                    stop=True,
                )

            # prefetch the permutation gather for the final block so its
            # store chain is not serialized behind an indirect DMA at the end.
            # NOTE: the hardware indirect-DMA path only handles one index per
            # partition (2-D dest APs) — multi-index gathers return garbage.
            glast = gi_pool.tile([128, I1, 512], BF16, tag="glast")
            for j1 in range(I1):
                nc.gpsimd.indirect_dma_start(
                    out=glast[:, j1, :],
                    out_offset=None,
                    in_=vT[:, :],
                    in_offset=bass.IndirectOffsetOnAxis(
                        ap=gi[:, J2B - 1, j1 : j1 + 1], axis=0
                    ),
                    element_offset=(MC - 1) * 512,
                )

            def emit_rep(first):
                # per-(i1 group, mc) value.T tiles [128, 4kt, 512]: one fat DMA
                # per matmul k-group so PE can start after the first ~1.5us.
                # mc=0 loads issue from SP, mc=1 from Pool (Act stays free to
                # drain PSUM from the first block on).
                # mc=0 loads get absolute DMA priority (SP queue, first);
                # mc=1 loads are interleaved into the block loop below so
                # they never compete with the pipeline ramp
                vts = [[None] * I1 for _ in range(MC)]
                for i1 in range(I1):
                    t = vt_pool.tile([128, KS, 512], BF16, tag=f"vt0_{i1}")
                    if first and i1 == 0:
                        # single-k-tile loads so the first matmul can
                        # start ~1.5us earlier
                        for ks in range(KS):
                            nc.sync.dma_start(
                                out=t[:, ks, :],
                                in_=vT[ks * 128 : (ks + 1) * 128, 0:512],
                            )
                    else:
                        vsrc = vT[i1 * B : (i1 + 1) * B, 0:512].rearrange(
                            "(k p) m -> p k m", p=128
                        )
                        nc.sync.dma_start(out=t, in_=vsrc)
                    vts[0][i1] = t
                for i1 in range(I1):
                    vts[1][i1] = vt_pool.tile(
                        [128, KS, 512], BF16, tag=f"vt1_{i1}", name=f"vt1_{i1}"
                    )

                def load_mc1(i1):
                    vsrc = vT[i1 * B : (i1 + 1) * B, 512:1024].rearrange(
                        "(k p) m -> p k m", p=128
                    )
                    nc.sync.dma_start(out=vts[1][i1], in_=vsrc)

                blk = 0
                for mc in range(MC):
                    m0 = mc * 512
                    for j2b in range(J2B):
                        n2s = slice(j2b * 128, (j2b + 1) * 128)
                        u = u_pool.tile([128, I1, 512], BF16, tag="u")
                        av = a_pool.tile([128, I1, 512], BF16, tag="a")
                        cv = c_pool.tile([128, I1, 512], BF16, tag="c")
                        for half in range(2):
                            for q in range(2):
                                # 2-bank PSUM tile per i1 pair; 4-deep pool so
                                # the PE never waits on the Act drain
                                if first and mc == 0 and j2b == 0 and half == 0 and q == 0:
                                    ps = wps
                                else:
                                    ps = ps_pool.tile(
                                        [128, 2, 512], mybir.dt.float32, tag="ps"
                                    )
                                for i2 in range(2):
                                    i1 = half * 4 + q * 2 + i2
                                    for ks in range(KS):
                                        nc.tensor.matmul(
                                            out=ps[:, i2, :],
                                            lhsT=hts[ks][:, n2s],
                                            rhs=vts[mc][i1][:, ks, :],
                                            start=(ks == 0),
                                            stop=(ks == KS - 1),
                                        )
                                qs = half * 4 + q * 2
                                nc.scalar.copy(
                                    out=u[:, qs : qs + 2, :], in_=ps[:, :, :]
                                )
                            # butterfly stages bit0 + bit1 on this half only,
                            # so DVE starts as soon as a quarter-pair drains
                            h0 = half * 4
                            uh = u[:, h0 : h0 + 4, :].rearrange(
                                "p (t s) m -> p t s m", s=2
                            )
                            ah = av[:, h0 : h0 + 4, :].rearrange(
                                "p (t s) m -> p t s m", s=2
                            )
                            # stage bit0 (pairs (2t, 2t+1)): u -> a
                            nc.vector.tensor_tensor(
                                out=ah[:, :, 0], in0=uh[:, :, 0], in1=uh[:, :, 1],
                                op=add,
                            )
                            nc.vector.tensor_tensor(
                                out=ah[:, :, 1], in0=uh[:, :, 0], in1=uh[:, :, 1],
                                op=sub,
                            )
                            # stage bit1 (pairs (i, i+2)): a -> u (reuse)
                            a2 = av[:, h0 : h0 + 4, :].rearrange(
                                "p (t s) m -> p t s m", t=2
                            )
                            u2 = u[:, h0 : h0 + 4, :].rearrange(
                                "p (t s) m -> p t s m", t=2
                            )
                            nc.vector.tensor_tensor(
                                out=u2[:, 0], in0=a2[:, 0], in1=a2[:, 1], op=add
                            )
                            nc.vector.tensor_tensor(
                                out=u2[:, 1], in0=a2[:, 0], in1=a2[:, 1], op=sub
                            )
                        # stage bit2 (pairs (i, i+4)): u -> c, then per output
                        # half: fused gather-add + store in j1 pairs, so the
                        # kernel tail is only a quarter block deep
                        osel = o[:, m0 : m0 + 512].rearrange(
                            "(j1 q p) m -> p j1 q m", j1=I1, q=J2B
                        )
                        last = mc == MC - 1 and j2b == J2B - 1
                        for oh, op_ in ((0, add), (1, sub)):
                            j1s = slice(oh * 4, (oh + 1) * 4)
                            beng = nc.gpsimd if (op_ is sub and not last) else nc.vector
                            beng.tensor_tensor(
                                out=cv[:, j1s, :],
                                in0=u[:, 0:4, :],
                                in1=u[:, 4:8, :],
                                op=op_,
                            )
                            if last:
                                # final block: gather was prefetched; add on
                                # DVE and store directly (short tail)
                                nc.vector.tensor_tensor(
                                    out=cv[:, j1s, :],
                                    in0=cv[:, j1s, :],
                                    in1=glast[:, j1s, :],
                                    op=add,
                                )
                                nc.sync.dma_start(
                                    out=osel[:, j1s, j2b, :], in_=cv[:, j1s, :]
                                )
                                continue
                            for qq in range(4):
                                j1 = oh * 4 + qq
                                # c[p, j1, m] += vT[src[j1*512+j2b*128+p], m0+m]
                                # (single-index 2-D gather: HW constraint)
                                nc.gpsimd.indirect_dma_start(
                                    out=cv[:, j1, :],
                                    out_offset=None,
                                    in_=vT[:, :],
                                    in_offset=bass.IndirectOffsetOnAxis(
                                        ap=gi[:, j2b, j1 : j1 + 1], axis=0
                                    ),
                                    element_offset=m0,
                                    compute_op=add,
                                )
                            j1s2 = slice(oh * 4, oh * 4 + 4)
                            nc.sync.dma_start(
                                out=osel[:, j1s2, j2b, :], in_=cv[:, j1s2, :]
                            )
                        # trickle in the mc=1 input behind this block's stores
                        if mc == 0 and blk < 4:
                            load_mc1(2 * blk)
                            load_mc1(2 * blk + 1)
                        blk += 1

            if hw_loop:
                # hardware loop: constant NEFF size regardless of reps, so a
                # wall-clock delta between rep counts measures device time
                with tc.For_i(0, reps, 1):
                    emit_rep(False)
            else:
                for rep in range(reps):
                    emit_rep(rep == 0)
    nc.compile()
    return nc


def make_in_maps_h(value, src):
    value = np.asarray(value, dtype=np.float32)
    Hs = np.ascontiguousarray(
        (_hadamard_pm1(B) / np.float32(64.0)).astype(ml_dtypes.bfloat16)
    )
    # gidx[p, q, j1] = src[j1*512 + q*128 + p]
    gidx = np.ascontiguousarray(
        src.reshape(I1, J2B, 128).transpose(2, 1, 0).astype(np.int32)
    )
    in_maps = []
    for c in range(N_CORES):
        vc = value[c * MPC : (c + 1) * MPC, :].astype(ml_dtypes.bfloat16)
        in_maps.append(
            {
                "vT": np.ascontiguousarray(vc.T),
                "hc": Hs,
                "gidx": gidx,
            }
        )
    return in_maps


def kernel(value, weight, permutation):
    value = np.asarray(value, dtype=np.float32)
    weight = np.asarray(weight, dtype=np.float32)
    permutation = np.asarray(permutation, dtype=np.float32)
    src = check_structure(weight, permutation)
    if src is not None:
        if "had" not in _cache:
            _cache["had"] = build_hadamard()
        nc = _cache["had"]
        in_maps = make_in_maps_h(value, src)
    else:
        if "dense" not in _cache:
            _cache["dense"] = build_dense()
        nc = _cache["dense"]
        in_maps = make_in_maps(value, weight, permutation)
    res = run_bass_kernel_spmd(nc, in_maps, core_ids=list(range(N_CORES)))
    out = np.empty((ROWS, N), dtype=np.float32)
    for c in range(N_CORES):
        out[c * MPC : (c + 1) * MPC, :] = (
            res.results[c]["o"].astype(np.float32).T
        )
    return out


# ---------------- dense fallback (unstructured weight/permutation) ----------------


def build_dense():
    nc = bacc.Bacc("TRN2", target_bir_lowering=False)
    vT = nc.dram_tensor("vT", (N, MPC), mybir.dt.float32r, kind="ExternalInput")
    wgt = nc.dram_tensor("wgt", (N, N), mybir.dt.float32, kind="ExternalInput")
    prm = nc.dram_tensor("prm", (N, N), mybir.dt.float32, kind="ExternalInput")
    o = nc.dram_tensor("o", (N, MPC), mybir.dt.float32, kind="ExternalOutput")

    with tile.TileContext(nc) as tc:
        with (
            tc.tile_pool(name="vt", bufs=1) as vt_pool,
            tc.tile_pool(name="wp", bufs=2) as wp_pool,
            tc.tile_pool(name="pp", bufs=2) as pp_pool,
            tc.tile_pool(name="ps", bufs=4, space="PSUM") as ps_pool,
            tc.tile_pool(name="os", bufs=4) as os_pool,
        ):
            vts = []
            for t in range(KT):
                vt_t = vt_pool.tile([128, MPC], mybir.dt.float32r, tag=f"vt{t}")
                nc.sync.dma_start(out=vt_t, in_=vT[t * 128 : (t + 1) * 128, :])
                vts.append(vt_t)

            for nb in range(NB):
                n0 = nb * 128
                wp = wp_pool.tile([128, KT, 128], mybir.dt.float32r, tag="wp")
                pp = pp_pool.tile([128, KT, 128], mybir.dt.float32, tag="pp")
                wsrc = wgt[:, n0 : n0 + 128].rearrange("(kt p) j -> p kt j", p=128)
                psrc = prm[:, n0 : n0 + 128].rearrange("(kt p) j -> p kt j", p=128)
                nc.sync.dma_start(out=wp[:, :, :].bitcast(mybir.dt.float32), in_=wsrc)
                nc.sync.dma_start(out=pp, in_=psrc)
                nc.vector.tensor_tensor(
                    out=wp[:, :, :],
                    in0=wp[:, :, :].bitcast(mybir.dt.float32),
                    in1=pp[:, :, :],
                    op=mybir.AluOpType.add,
                )
                for mc in range(MC):
                    ps = ps_pool.tile([128, 512], mybir.dt.float32, tag="ps")
                    for kt in range(KT):
                        nc.tensor.matmul(
                            out=ps[:, :],
                            lhsT=wp[:, kt, :],
                            rhs=vts[kt][:, mc * 512 : (mc + 1) * 512],
                            start=(kt == 0),
                            stop=(kt == KT - 1),
                        )
                    ot = os_pool.tile([128, 512], mybir.dt.float32, tag="os")
                    nc.scalar.copy(out=ot[:, :], in_=ps[:, :])
                    nc.sync.dma_start(
                        out=o[n0 : n0 + 128, mc * 512 : (mc + 1) * 512], in_=ot
                    )
    nc.compile()
    return nc


def make_in_maps(value, weight, permutation):
    vT = np.ascontiguousarray(np.asarray(value, dtype=np.float32).T)
    w = np.ascontiguousarray(weight, dtype=np.float32)
    p = np.ascontiguousarray(permutation, dtype=np.float32)
    in_maps = []
    for c in range(N_CORES):
        in_maps.append(
            {
                "vT": np.ascontiguousarray(vT[:, c * MPC : (c + 1) * MPC]),
                "wgt": w,
                "prm": p,
            }
        )
    return in_maps


# revision 16
# speedup vs baseline: 1.0057x; 1.0057x over previous
"""Trainium2 Bass kernel for nn_HadamardTransform: out = value @ (weight + permutation).

weight is the scaled Sylvester Hadamard H4096/64, permutation is one-hot.
Data-parallel over token rows across 8 cores; each core computes, in the
transposed frame, o[n, m] = sum_k Hs[k, n] vT[k, m] + vT[src[n], m].

Decomposition Hs = H8 (x) H512 (Sylvester Kronecker):
  - PE: per k1-group i1 in 0..7, U_i1[n2, m] = sum_k2 (H512/64)[k2, n2] vT[i1*512+k2, m]
        (bf16 lhsT/rhs, fp32 PSUM, 4 accumulating matmuls per tile)
  - Act: PSUM -> SBUF drain with fp32 -> bf16 downcast (2-bank copies)
  - DVE: 3 radix-2 butterfly stages over the i1 axis in bf16 (2x DVE mode);
         one op per block offloaded to Pool.
  - Pool/SWDGE: per-output-row indirect gathers vT[src[n], m0:m0+512] fused
    with the final add via DMA compute (cce add).  The HW indirect-DMA path
    only supports one index per partition (2-D dest APs), so gathers are
    issued per j1 row.
Everything is bf16 on the wire: ~24.5 MB of DMA per core vs 52 MB for fp32,
which puts the kernel at the 360 GB/s-per-core DMA roofline (~68 us) with
PE (~57 us) and DVE (~52 us) hidden underneath.
"""

import sys

sys.path.insert(0, "/opt/trn_rl_repo")

import ml_dtypes
import numpy as np

import concourse.bacc as bacc
import concourse.bass as bass
import concourse.mybir as mybir
import concourse.tile as tile
from concourse.bass_utils import run_bass_kernel_spmd

ROWS = 8192
N = 4096
N_CORES = 8
MPC = ROWS // N_CORES  # 1024 token rows per core
KT = N // 128  # 32 k-tiles
NB = N // 128  # 32 n-blocks
MC = MPC // 512  # 2 m-chunks

I1 = 8  # radix handled by DVE butterflies
B = N // I1  # 512-point transform on the PE
KS = B // 128  # 4 k-subtiles per i1 group
J2B = B // 128  # 4 n2 blocks

BF16 = mybir.dt.bfloat16

_cache = {}


def _hadamard_pm1(n):
    idx = np.arange(n, dtype=np.int64)
    m = idx[:, None] & idx[None, :]
    pop = np.zeros_like(m)
    for _ in range(int(np.log2(n))):
        pop += m & 1
        m >>= 1
    return np.where(pop % 2 == 0, 1.0, -1.0).astype(np.float32)


def check_structure(weight, permutation):
    """weight must be the scaled Sylvester Hadamard, permutation one-hot."""
    H = _hadamard_pm1(N) / np.sqrt(np.float32(N))
    if not np.array_equal(weight, H):
        return None
    src = np.argmax(permutation, axis=0).astype(np.int32)
    ok = (
        permutation[src, np.arange(N)].min() == 1.0
        and permutation.sum() == N
        and np.abs(permutation).sum() == N
    )
    return src if ok else None


def build_hadamard(reps=1, hw_loop=False):
    nc = bacc.Bacc("TRN2", target_bir_lowering=False)
    vT = nc.dram_tensor("vT", (N, MPC), BF16, kind="ExternalInput")
    hc = nc.dram_tensor("hc", (B, B), BF16, kind="ExternalInput")
    gidx = nc.dram_tensor("gidx", (128, J2B, I1), mybir.dt.int32, kind="ExternalInput")
    o = nc.dram_tensor("o", (N, MPC), BF16, kind="ExternalOutput")

    add, sub = mybir.AluOpType.add, mybir.AluOpType.subtract

    with tile.TileContext(nc) as tc:
        with (
            tc.tile_pool(name="h", bufs=1) as h_pool,
            tc.tile_pool(name="gi", bufs=1) as gi_pool,
            tc.tile_pool(name="vt", bufs=1) as vt_pool,
            tc.tile_pool(name="ps", bufs=4, space="PSUM") as ps_pool,
            tc.tile_pool(name="u", bufs=3) as u_pool,
            tc.tile_pool(name="a", bufs=3) as a_pool,
            tc.tile_pool(name="c", bufs=3) as c_pool,
        ):
            # H and index loads go on the Pool queue (Act's first instruction
            # triggers a slow activation-table load; SP streams value.T at t=0)
            hts = []
            for ks in range(KS):
                ht = h_pool.tile([128, B], BF16, tag=f"h{ks}")
                nc.gpsimd.dma_start(out=ht, in_=hc[ks * 128 : (ks + 1) * 128, :])
                hts.append(ht)
            gi = gi_pool.tile([128, J2B, I1], mybir.dt.int32, tag="gi")
            nc.gpsimd.dma_start(out=gi, in_=gidx[:, :, :])

            # warm the PE out of its p-state ramp while the input streams in;
            # the warmup writes into block 0's first PSUM tile (overwritten by
            # the real accumulation) so no PSUM rotation slot is wasted
            wps = ps_pool.tile([128, 2, 512], mybir.dt.float32, tag="ps")
            for w in range(7):
                nc.tensor.matmul(
                    out=wps[:, 0, :],
                    lhsT=hts[0][:, 0:128],
                    rhs=hts[0][:, :],
                    start=True,
                    stop=True,
                )

            # prefetch the permutation gather for the final block so its
            # store chain is not serialized behind an indirect DMA at the end.
            # NOTE: the hardware indirect-DMA path only handles one index per
            # partition (2-D dest APs) — multi-index gathers return garbage.
            glast = gi_pool.tile([128, I1, 512], BF16, tag="glast")
            for j1 in range(I1):
                nc.gpsimd.indirect_dma_start(
                    out=glast[:, j1, :],
                    out_offset=None,
                    in_=vT[:, :],
                    in_offset=bass.IndirectOffsetOnAxis(
                        ap=gi[:, J2B - 1, j1 : j1 + 1], axis=0
                    ),
                    element_offset=(MC - 1) * 512,
                )

            def emit_rep(first):
                # per-(i1 group, mc) value.T tiles [128, 4kt, 512]: one fat DMA
                # per matmul k-group so PE can start after the first ~1.5us.
                # mc=0 loads issue from SP, mc=1 from Pool (Act stays free to
                # drain PSUM from the first block on).
                # mc=0 loads get absolute DMA priority (SP queue, first);
                # mc=1 loads are interleaved into the block loop below so
                # they never compete with the pipeline ramp
                vts = [[None] * I1 for _ in range(MC)]
                for i1 in range(I1):
                    t = vt_pool.tile([128, KS, 512], BF16, tag=f"vt0_{i1}")
                    if first and i1 == 0:
                        # single-k-tile loads so the first matmul can
                        # start ~1.5us earlier
                        for ks in range(KS):
                            nc.sync.dma_start(
                                out=t[:, ks, :],
                                in_=vT[ks * 128 : (ks + 1) * 128, 0:512],
                            )
                    else:
                        for kh in range(2):
                            vsrc = vT[
                                (i1 * KS + kh * 2) * 128 : (i1 * KS + kh * 2 + 2) * 128,
                                0:512,
                            ].rearrange("(k p) m -> p k m", p=128)
                            nc.sync.dma_start(out=t[:, kh * 2 : kh * 2 + 2, :], in_=vsrc)
                    vts[0][i1] = t
                for i1 in range(I1):
                    vts[1][i1] = vt_pool.tile(
                        [128, KS, 512], BF16, tag=f"vt1_{i1}", name=f"vt1_{i1}"
                    )

                def load_mc1(i1):
                    vsrc = vT[i1 * B : (i1 + 1) * B, 512:1024].rearrange(
                        "(k p) m -> p k m", p=128
                    )
                    nc.sync.dma_start(out=vts[1][i1], in_=vsrc)

                blk = 0
                for mc in range(MC):
                    m0 = mc * 512
                    for j2b in range(J2B):
                        n2s = slice(j2b * 128, (j2b + 1) * 128)
                        u = u_pool.tile([128, I1, 512], BF16, tag="u")
                        av = a_pool.tile([128, I1, 512], BF16, tag="a")
                        cv = c_pool.tile([128, I1, 512], BF16, tag="c")
                        for half in range(2):
                            for q in range(2):
                                # 2-bank PSUM tile per i1 pair; 4-deep pool so
                                # the PE never waits on the Act drain
                                if first and mc == 0 and j2b == 0 and half == 0 and q == 0:
                                    ps = wps
                                else:
                                    ps = ps_pool.tile(
                                        [128, 2, 512], mybir.dt.float32, tag="ps"
                                    )
                                for i2 in range(2):
                                    i1 = half * 4 + q * 2 + i2
                                    for ks in range(KS):
                                        nc.tensor.matmul(
                                            out=ps[:, i2, :],
                                            lhsT=hts[ks][:, n2s],
                                            rhs=vts[mc][i1][:, ks, :],
                                            start=(ks == 0),
                                            stop=(ks == KS - 1),
                                        )
                                qs = half * 4 + q * 2
                                nc.scalar.copy(
                                    out=u[:, qs : qs + 2, :], in_=ps[:, :, :]
                                )
                            # butterfly stages bit0 + bit1 on this half only,
                            # so DVE starts as soon as a quarter-pair drains
                            h0 = half * 4
                            uh = u[:, h0 : h0 + 4, :].rearrange(
                                "p (t s) m -> p t s m", s=2
                            )
                            ah = av[:, h0 : h0 + 4, :].rearrange(
                                "p (t s) m -> p t s m", s=2
                            )
                            # stage bit0 (pairs (2t, 2t+1)): u -> a
                            nc.vector.tensor_tensor(
                                out=ah[:, :, 0], in0=uh[:, :, 0], in1=uh[:, :, 1],
                                op=add,
                            )
                            nc.vector.tensor_tensor(
                                out=ah[:, :, 1], in0=uh[:, :, 0], in1=uh[:, :, 1],
                                op=sub,
                            )
                            # stage bit1 (pairs (i, i+2)): a -> u (reuse)
                            a2 = av[:, h0 : h0 + 4, :].rearrange(
                                "p (t s) m -> p t s m", t=2
                            )
                            u2 = u[:, h0 : h0 + 4, :].rearrange(
                                "p (t s) m -> p t s m", t=2
                            )
                            nc.vector.tensor_tensor(
                                out=u2[:, 0], in0=a2[:, 0], in1=a2[:, 1], op=add
                            )
                            nc.vector.tensor_tensor(
                                out=u2[:, 1], in0=a2[:, 0], in1=a2[:, 1], op=sub
                            )
                        # stage bit2 (pairs (i, i+4)): u -> c, then per output
                        # half: fused gather-add + store in j1 pairs, so the
                        # kernel tail is only a quarter block deep
                        osel = o[:, m0 : m0 + 512].rearrange(
                            "(j1 q p) m -> p j1 q m", j1=I1, q=J2B
                        )
                        last = mc == MC - 1 and j2b == J2B - 1
                        for oh, op_ in ((0, add), (1, sub)):
                            j1s = slice(oh * 4, (oh + 1) * 4)
                            beng = nc.gpsimd if (op_ is sub and not last) else nc.vector
                            beng.tensor_tensor(
                                out=cv[:, j1s, :],
                                in0=u[:, 0:4, :],
                                in1=u[:, 4:8, :],
                                op=op_,
                            )
                            if last:
                                # final block: gather was prefetched; add on
                                # DVE and store directly (short tail)
                                nc.vector.tensor_tensor(
                                    out=cv[:, j1s, :],
                                    in0=cv[:, j1s, :],
                                    in1=glast[:, j1s, :],
                                    op=add,
                                )
                                nc.sync.dma_start(
                                    out=osel[:, j1s, j2b, :], in_=cv[:, j1s, :]
                                )
                                continue
                            for qq in range(4):
                                j1 = oh * 4 + qq
                                # c[p, j1, m] += vT[src[j1*512+j2b*128+p], m0+m]
                                # (single-index 2-D gather: HW constraint)
                                nc.gpsimd.indirect_dma_start(
                                    out=cv[:, j1, :],
                                    out_offset=None,
                                    in_=vT[:, :],
                                    in_offset=bass.IndirectOffsetOnAxis(
                                        ap=gi[:, j2b, j1 : j1 + 1], axis=0
                                    ),
                                    element_offset=m0,
                                    compute_op=add,
                                )
                            j1s2 = slice(oh * 4, oh * 4 + 4)
                            nc.sync.dma_start(
                                out=osel[:, j1s2, j2b, :], in_=cv[:, j1s2, :]
                            )
                        # trickle in the mc=1 input behind this block's stores
                        if mc == 0 and blk < 4:
                            load_mc1(2 * blk)
                            load_mc1(2 * blk + 1)
                        blk += 1

            if hw_loop:
                # hardware loop: constant NEFF size regardless of reps, so a
                # wall-clock delta between rep counts measures device time
                with tc.For_i(0, reps, 1):
                    emit_rep(False)
            else:
                for rep in range(reps):
                    emit_rep(rep == 0)
    nc.compile()
    return nc


def make_in_maps_h(value, src):
    value = np.asarray(value, dtype=np.float32)
    Hs = np.ascontiguousarray(
        (_hadamard_pm1(B) / np.float32(64.0)).astype(ml_dtypes.bfloat16)
    )
    # gidx[p, q, j1] = src[j1*512 + q*128 + p]
    gidx = np.ascontiguousarray(
        src.reshape(I1, J2B, 128).transpose(2, 1, 0).astype(np.int32)
    )
    in_maps = []
    for c in range(N_CORES):
        vc = value[c * MPC : (c + 1) * MPC, :].astype(ml_dtypes.bfloat16)
        in_maps.append(
            {
                "vT": np.ascontiguousarray(vc.T),
                "hc": Hs,
                "gidx": gidx,
            }
        )
    return in_maps


def kernel(value, weight, permutation):
    value = np.asarray(value, dtype=np.float32)
    weight = np.asarray(weight, dtype=np.float32)
    permutation = np.asarray(permutation, dtype=np.float32)
    src = check_structure(weight, permutation)
    if src is not None:
        if "had" not in _cache:
            _cache["had"] = build_hadamard()
        nc = _cache["had"]
        in_maps = make_in_maps_h(value, src)
    else:
        if "dense" not in _cache:
            _cache["dense"] = build_dense()
        nc = _cache["dense"]
        in_maps = make_in_maps(value, weight, permutation)
    res = run_bass_kernel_spmd(nc, in_maps, core_ids=list(range(N_CORES)))
    out = np.empty((ROWS, N), dtype=np.float32)
    for c in range(N_CORES):
        out[c * MPC : (c + 1) * MPC, :] = (
            res.results[c]["o"].astype(np.float32).T
        )
    return out


# ---------------- dense fallback (unstructured weight/permutation) ----------------


def build_dense():
    nc = bacc.Bacc("TRN2", target_bir_lowering=False)
    vT = nc.dram_tensor("vT", (N, MPC), mybir.dt.float32r, kind="ExternalInput")
    wgt = nc.dram_tensor("wgt", (N, N), mybir.dt.float32, kind="ExternalInput")
    prm = nc.dram_tensor("prm", (N, N), mybir.dt.float32, kind="ExternalInput")
    o = nc.dram_tensor("o", (N, MPC), mybir.dt.float32, kind="ExternalOutput")

    with tile.TileContext(nc) as tc:
        with (
            tc.tile_pool(name="vt", bufs=1) as vt_pool,
            tc.tile_pool(name="wp", bufs=2) as wp_pool,
            tc.tile_pool(name="pp", bufs=2) as pp_pool,
            tc.tile_pool(name="ps", bufs=4, space="PSUM") as ps_pool,
            tc.tile_pool(name="os", bufs=4) as os_pool,
        ):
            vts = []
            for t in range(KT):
                vt_t = vt_pool.tile([128, MPC], mybir.dt.float32r, tag=f"vt{t}")
                nc.sync.dma_start(out=vt_t, in_=vT[t * 128 : (t + 1) * 128, :])
                vts.append(vt_t)

            for nb in range(NB):
                n0 = nb * 128
                wp = wp_pool.tile([128, KT, 128], mybir.dt.float32r, tag="wp")
                pp = pp_pool.tile([128, KT, 128], mybir.dt.float32, tag="pp")
                wsrc = wgt[:, n0 : n0 + 128].rearrange("(kt p) j -> p kt j", p=128)
                psrc = prm[:, n0 : n0 + 128].rearrange("(kt p) j -> p kt j", p=128)
                nc.sync.dma_start(out=wp[:, :, :].bitcast(mybir.dt.float32), in_=wsrc)
                nc.sync.dma_start(out=pp, in_=psrc)
                nc.vector.tensor_tensor(
                    out=wp[:, :, :],
                    in0=wp[:, :, :].bitcast(mybir.dt.float32),
                    in1=pp[:, :, :],
                    op=mybir.AluOpType.add,
                )
                for mc in range(MC):
                    ps = ps_pool.tile([128, 512], mybir.dt.float32, tag="ps")
                    for kt in range(KT):
                        nc.tensor.matmul(
                            out=ps[:, :],
                            lhsT=wp[:, kt, :],
                            rhs=vts[kt][:, mc * 512 : (mc + 1) * 512],
                            start=(kt == 0),
                            stop=(kt == KT - 1),
                        )
                    ot = os_pool.tile([128, 512], mybir.dt.float32, tag="os")
                    nc.scalar.copy(out=ot[:, :], in_=ps[:, :])
                    nc.sync.dma_start(
                        out=o[n0 : n0 + 128, mc * 512 : (mc + 1) * 512], in_=ot
                    )
    nc.compile()
    return nc


def make_in_maps(value, weight, permutation):
    vT = np.ascontiguousarray(np.asarray(value, dtype=np.float32).T)
    w = np.ascontiguousarray(weight, dtype=np.float32)
    p = np.ascontiguousarray(permutation, dtype=np.float32)
    in_maps = []
    for c in range(N_CORES):
        in_maps.append(
            {
                "vT": np.ascontiguousarray(vT[:, c * MPC : (c + 1) * MPC]),
                "wgt": w,
                "prm": p,
            }
        )
    return in_maps
